# revision 1
# baseline (speedup 1.0000x reference)
"""Deformable temporal conv1d (kernel (1,3), stride 1, pad 1) on 8 TRN2 cores.

v3: transposed layout, fp16 datapath, shift-matmul accumulation.

Layout: W=128 on partitions, free dims (c, h).  Per-pixel modulation maps
A_t[w, h] broadcast along c via stride-0 APs (no partition broadcast).
z_j staged as fp16 base slabs [w, c, srows] aligned at image col 0
(zb_j[w'] = W_j @ x[:, :, w']).  A term (j,dh,dw) needs z at col w+j-1+dw
= base col w+s', s'=j-1+dw in [-2,2].  The partition shift s' is folded
into the PSUM accumulation: products tmp[w'] = A_t[w'-s'] * zb_j[w',h+dh]
are accumulated with lhsT = eye(k=-s') so acc[w] += tmp[w+s'].  The
shifted A rows come from tiny PE shift-matmuls on [w, 3, blk] A-map
groups.  Out-of-range columns are image zero-padding, so no edge cases.
Vertical shifts (dh) are free-dim offsets.  27 products on DVE (fp16,
2x mode), 27x4 accumulation matmuls on the otherwise idle PE, staging
and activation maps on Act.  |offset| in (1,2) handled by an If-gated
36-term outer-ring pass (PE/Act/DVE, no DMA, exact for |off|<2).
Output written as [w, h, c]; the host transposes back to [c, h, w].

Sharding: core i handles batch b=i//2, H-half hh=i%2 (256 output rows).
"""

import numpy as np
from contextlib import ExitStack

import concourse.bass as bass
import concourse.bacc as bacc
import concourse.tile as tile
import concourse.mybir as mybir
from concourse.bass_utils import run_bass_kernel_spmd

F32 = mybir.dt.float32
F16 = mybir.dt.float16
U32 = mybir.dt.uint32
AF = mybir.ActivationFunctionType
OP = mybir.AluOpType
ET = mybir.EngineType

B, C, H, W = 4, 128, 512, 128
NCORES = 8
ROWS = H // 2          # output rows per core
KTAP = 3
HALO = 2               # halo rows each side
COL0 = 3               # x column offset of image col 0
PITCH = W + 8          # 3 zero cols left, 5 right

# main terms: t = ih*9 + iw*3 + j  for dh,dw in {-1,0,1}, j in 0..2
TERMS = [(ih * 9 + iw * 3 + j, j, dh, dw)
         for ih, dh in enumerate((-1, 0, 1))
         for iw, dw in enumerate((-1, 0, 1))
         for j in range(KTAP)]
# main-pass A-shift groups (iw, j) with s' = (j-1)+(iw-1) != 0
ACOMBO = [(iw, j) for iw in range(3) for j in range(KTAP)
          if (j - 1) + (iw - 1) != 0]
ACI = {c: i for i, c in enumerate(ACOMBO)}
# ring combos, grouped so rows for fixed (dw, j) are amap-stride-3 slices
RING = [(dh, dw) for dw in (-1, 0, 1) for dh in (-2, 2)] + \
       [(dh, dw) for dw in (-2, 2) for dh in (-1, 0, 1)]


def build_nc(rows=ROWS, blk=16, ring=True):
    assert rows % blk == 0
    nb = rows // blk
    srows = blk + 2 * HALO                    # 20
    hb = blk // 2

    nc = bacc.Bacc()
    x_d = nc.declare_dram_parameter("x", [C, rows + 2 * HALO, PITCH], F16,
                                    isOutput=False)
    wz_d = nc.declare_dram_parameter("wz", [C, KTAP * C], F16, isOutput=False)
    wom_d = nc.declare_dram_parameter("wom", [C, KTAP * 9], F16,
                                      isOutput=False)
    obr_d = nc.declare_dram_parameter("obr", [1, 9], F16, isOutput=False)
    b5_d = nc.declare_dram_parameter("b5", [W, 5], F32, isOutput=False)
    es_d = nc.declare_dram_parameter("esh", [W, 7 * W], F16, isOutput=False)
    ones1_d = nc.declare_dram_parameter("ones1", [1, W], F16, isOutput=False)
    cbh_d = nc.declare_dram_parameter("cbh", [1, C * 8], F16,
                                      isOutput=False)
    out_d = nc.declare_dram_parameter("out", [W, rows * C], F32,
                                      isOutput=True)

    with tile.TileContext(nc) as tc, ExitStack() as ctx:
        cpool = ctx.enter_context(tc.tile_pool(name="consts", bufs=1))
        xpool = ctx.enter_context(tc.tile_pool(name="xs", bufs=3))
        ompool = ctx.enter_context(tc.tile_pool(name="om", bufs=3))
        mpool = ctx.enter_context(tc.tile_pool(name="maps", bufs=3))
        fpool = ctx.enter_context(tc.tile_pool(name="flag", bufs=3))
        tpool = ctx.enter_context(tc.tile_pool(name="tmp", bufs=3))
        spool = ctx.enter_context(tc.tile_pool(name="ostg", bufs=2))
        ps_z = ctx.enter_context(
            tc.tile_pool(name="ps_z", bufs=2, space="PSUM"))
        ps_om = ctx.enter_context(
            tc.tile_pool(name="ps_om", bufs=1, space="PSUM"))
        ps_ash = ctx.enter_context(
            tc.tile_pool(name="ps_ash", bufs=1, space="PSUM"))
        ps_a = ctx.enter_context(
            tc.tile_pool(name="ps_a", bufs=1, space="PSUM"))

        # constants
        wz = cpool.tile([C, KTAP * C], F16, tag="wz")
        nc.sync.dma_start(wz[:], wz_d[:])
        wom = cpool.tile([C, KTAP * 9], F16, tag="wom")
        nc.sync.dma_start(wom[:], wom_d[:])
        obr = cpool.tile([1, 9], F16, tag="obr")
        nc.sync.dma_start(obr[:], obr_d[:])
        # b5 columns hold -dlt for dlt in (-2,-1,0,1,2): (2,1,0,-1,-2)
        b5 = cpool.tile([W, 5], F32, tag="b5")
        nc.sync.dma_start(b5[:], b5_d[:])
        # esh[:, k+3, :] = eye(W, k)
        esh = cpool.tile([W, 7, W], F16, tag="esh")
        nc.sync.dma_start(esh[:].rearrange("p a b -> p (a b)"), es_d[:])
        ones1 = cpool.tile([1, W], F16, tag="ones1")
        nc.sync.dma_start(ones1[:], ones1_d[:])
        cbh = cpool.tile([1, C, hb], F16, tag="cbh")
        nc.sync.dma_start(cbh[:].rearrange("p a b -> p (a b)"), cbh_d[:])

        # persistent base z slabs [j] -> [W, C, srows], x2 parity
        zs_bufs = []
        for i in range(2):
            zrow = []
            for j in range(KTAP):
                z = cpool.tile([W, C, srows], F16, tag=f"z{i}_{j}")
                zrow.append(z)
            zs_bufs.append(zrow)

        def maps_phase(b):
            """x DMA, om conv, activation maps, flag, A maps, A shifts."""
            r0 = b * blk
            st = {}
            xs = xpool.tile([C, srows, PITCH], F16, tag="xs")
            nc.sync.dma_start(xs[:], x_d[:, r0:r0 + srows, :])
            st["xs"] = xs

            # offset/mask conv: om [W, blk, 9] f32 (ob bias via ones row)
            om = ompool.tile([W, blk, 9], F32, tag="om")
            for rq in range(blk // 4):
                ps = ps_om.tile([W, 4, 9], F32, tag="psom")
                for ri in range(4):
                    r = HALO + rq * 4 + ri
                    nc.tensor.matmul(ps[:, ri, :], ones1[:], obr[:],
                                     start=True, stop=False)
                    for t in range(KTAP):
                        nc.tensor.matmul(
                            ps[:, ri, :],
                            xs[:, r, COL0 - 1 + t:COL0 - 1 + t + W],
                            wom[:, t * 9:(t + 1) * 9],
                            start=False, stop=(t == KTAP - 1))
                nc.scalar.activation(om[:, rq * 4:(rq + 1) * 4, :], ps[:],
                                     AF.Identity)

            # maps [W, 3j, blk] f16
            dyv = om[:, :, 0:6:2].transpose([0, 2, 1])
            dxv = om[:, :, 1:7:2].transpose([0, 2, 1])
            mskv = om[:, :, 6:9].transpose([0, 2, 1])
            st["dyv"], st["dxv"] = dyv, dxv
            msk = mpool.tile([W, 3, blk], F16, tag="msk")
            nc.scalar.activation(msk[:], mskv, AF.Sigmoid)
            st["msk"] = msk
            wyall = mpool.tile([W, 3, 3, blk], F16, tag="wyall")
            wxall = mpool.tile([W, 3, 3, blk], F16, tag="wxall")
            wy = {}
            wx = {}
            ay0 = None
            ax0 = None
            for i, (bi, dlt) in enumerate(((1, -1.0), (2, 0.0), (3, 1.0))):
                nbias = b5[:, bi:bi + 1]
                ayt = mpool.tile([W, 3, blk], F16, tag=f"ay{dlt}")
                nc.scalar.activation(ayt[:], dyv, AF.Abs, bias=nbias)
                nc.scalar.activation(wyall[:, i, :, :], ayt[:], AF.Relu,
                                     bias=1.0, scale=-1.0)
                wy[dlt] = wyall[:, i, :, :]
                axt = mpool.tile([W, 3, blk], F16, tag=f"ax{dlt}")
                nc.scalar.activation(axt[:], dxv, AF.Abs, bias=nbias)
                nc.scalar.activation(wxall[:, i, :, :], axt[:], AF.Relu,
                                     bias=1.0, scale=-1.0)
                wx[dlt] = wxall[:, i, :, :]
                if dlt == 0.0:
                    ay0, ax0 = ayt, axt
            st["wy"], st["wx"] = wy, wx

            # ring flag: any |dy|>1 or |dx|>1 in this block?
            if ring:
                mxf = fpool.tile([W, 3 * blk], F32, tag="mxf")
                nc.vector.tensor_tensor(
                    mxf[:], ay0[:].rearrange("p a b -> p (a b)"),
                    ax0[:].rearrange("p a b -> p (a b)"), op=OP.max)
                rmx = fpool.tile([W, 1], F32, tag="rmx")
                nc.vector.reduce_max(rmx[:], mxf[:],
                                     axis=mybir.AxisListType.X)
                rmxT = fpool.tile([1, W], F32, tag="rmxT")
                nc.sync.dma_start(rmxT[:], rmx[:])
                rfl = fpool.tile([1, 1], F32, tag="rfl")
                nc.vector.reduce_max(rfl[:], rmxT[:],
                                     axis=mybir.AxisListType.X)
                rfl01 = fpool.tile([1, 1], F32, tag="rfl01")
                nc.vector.tensor_scalar(rfl01[:], rfl[:], 1.0, None,
                                        op0=OP.is_gt)
                st["rfl01"] = rfl01

            # A maps [W, 27+36, blk] f16
            amap = mpool.tile([W, 27 + 36, blk], F16, tag="amap")
            st["amap"] = amap
            mywall = mpool.tile([W, 3, 3, blk], F16, tag="mywall")
            nc.vector.tensor_tensor(
                mywall[:], msk[:].unsqueeze(1).broadcast_to([W, 3, 3, blk]),
                wyall[:], op=OP.mult)
            myw = {dh: mywall[:, i, :, :]
                   for i, dh in enumerate((-1.0, 0.0, 1.0))}
            st["myw"] = myw
            for ih, dh in enumerate((-1.0, 0.0, 1.0)):
                nc.vector.tensor_tensor(
                    amap[:, ih * 9:(ih + 1) * 9, :]
                    .rearrange("p (a b) c -> p a b c", a=3),
                    mywall[:, ih:ih + 1, :, :]
                    .broadcast_to([W, 3, 3, blk]),
                    wxall[:], op=OP.mult)

            # A-shift matmuls: ashm[:, ci, ih, :] = A_(ih,iw,j)[w - s']
            aps = ps_ash.tile([W, 30, blk], F32, tag="ashp")
            for ci, (iw, j) in enumerate(ACOMBO):
                sp = (j - 1) + (iw - 1)
                t0 = iw * 3 + j
                nc.tensor.matmul(
                    aps[:, 3 * ci:3 * ci + 3, :], esh[:, sp + 3, :],
                    amap[:, t0:t0 + 19:9, :], start=True, stop=True)
            ashm = mpool.tile([W, 6, 3, blk], F16, tag="ashm")
            nc.scalar.activation(
                ashm[:].rearrange("p a b c -> p (a b) c"),
                aps[:, 0:18, :], AF.Identity)
            st["ashm"] = ashm
            return st

        def zconv_phase(b, st):
            """z convs (3 base slabs, col-0 aligned) + fp16 staging."""
            xs = st["xs"]
            zt = zs_bufs[b % 2]
            st["zt"] = zt
            for j in range(KTAP):
                zb = zt[j]
                for rq in range(srows // 4):
                    ps = ps_z.tile([W, 4, C], F32, tag="psz")
                    for ri in range(4):
                        nc.tensor.matmul(
                            ps[:, ri, :],
                            xs[:, rq * 4 + ri, COL0:COL0 + W],
                            wz[:, j * C:(j + 1) * C],
                            start=True, stop=True)
                    nc.scalar.activation(
                        zb[:, :, rq * 4:rq * 4 + 4],
                        ps[:].transpose([0, 2, 1]), AF.Identity)

        def accum_phase(b, st):
            """bias + 27 products (DVE) + shift-matmul PSUM accumulation."""
            zt = st["zt"]
            amap = st["amap"]
            ashm = st["ashm"]
            acc_a = ps_a.tile([W, C, hb], F32, tag="acca")
            acc_b = ps_a.tile([W, C, hb], F32, tag="accb")
            accs = [acc_a, acc_b]
            nt = len(TERMS)
            for hf in range(2):
                for cf in range(2):
                    cs = cf * (C // 2)
                    nc.tensor.matmul(
                        accs[hf][:, cs:cs + C // 2, :], ones1[:],
                        cbh[:, cs:cs + C // 2, :],
                        start=True, stop=False)
            for ti, (t, j, dh, dw) in enumerate(TERMS):
                sp = (j - 1) + dw
                zsrc = zt[j][:, :, HALO + dh:HALO + dh + blk]
                if sp == 0:
                    a_b = amap[:, t:t + 1, :].broadcast_to([W, C, blk])
                else:
                    a_b = ashm[:, ACI[(dw + 1, j)], dh + 1:dh + 2, :] \
                        .broadcast_to([W, C, blk])
                tmp = tpool.tile([W, C, blk], F16, tag="tmp")
                nc.vector.tensor_tensor(tmp[:], a_b, zsrc, op=OP.mult)
                lhs = esh[:, 3 - sp, :]
                for hf in range(2):
                    for cf in range(2):
                        nc.tensor.matmul(
                            accs[hf][:, cf * (C // 2):(cf + 1) * (C // 2), :],
                            lhs,
                            tmp[:, cf * (C // 2):(cf + 1) * (C // 2),
                                hf * hb:(hf + 1) * hb],
                            start=False, stop=(ti == nt - 1))

            # ring pass (rare): 36 extra terms, If-gated (PE/Act/DVE)
            if ring:
                dyv, dxv = st["dyv"], st["dxv"]
                msk, wy, wx, myw = st["msk"], st["wy"], st["wx"], st["myw"]
                flag_regs = []
                for et in (ET.PE, ET.Activation, ET.DVE):
                    eng = nc.engines[et]
                    r = eng.alloc_register(f"ringflag{b}")
                    eng.reg_load(r, st["rfl01"][:].bitcast(U32))
                    flag_regs.append(r)
                cond = nc.snap(bass.RegisterHandles(flag_regs), donate=True)
                with tc.If(cond != 0):
                    for bi, dlt in ((0, -2.0), (4, 2.0)):
                        nbias = b5[:, bi:bi + 1]
                        ayt = mpool.tile([W, 3, blk], F16, tag=f"ray{dlt}")
                        nc.scalar.activation(ayt[:], dyv, AF.Abs, bias=nbias)
                        wyt = mpool.tile([W, 3, blk], F16, tag=f"rwy{dlt}")
                        nc.scalar.activation(wyt[:], ayt[:], AF.Relu,
                                             bias=1.0, scale=-1.0)
                        wy[dlt] = wyt
                        axt = mpool.tile([W, 3, blk], F16, tag=f"rax{dlt}")
                        nc.scalar.activation(axt[:], dxv, AF.Abs, bias=nbias)
                        wxt = mpool.tile([W, 3, blk], F16, tag=f"rwx{dlt}")
                        nc.scalar.activation(wxt[:], axt[:], AF.Relu,
                                             bias=1.0, scale=-1.0)
                        wx[dlt] = wxt
                        mywt = mpool.tile([W, 3, blk], F16, tag=f"rmyw{dlt}")
                        nc.vector.tensor_tensor(mywt[:], msk[:], wy[dlt][:],
                                                op=OP.mult)
                        myw[dlt] = mywt
                    for ti, (dh, dw) in enumerate(RING):
                        t3 = 27 + ti * 3
                        nc.vector.tensor_tensor(
                            amap[:, t3:t3 + 3, :], myw[float(dh)][:],
                            wx[float(dw)][:], op=OP.mult)
                    # ring A-shifts
                    rps = ps_ash.tile([W, 30, blk], F32, tag="ashp")
                    rashm = mpool.tile([W, 30, blk], F16, tag="rashm")
                    arow = {}
                    row = 0
                    for dwi, dw in enumerate((-1, 0, 1)):
                        for j in range(KTAP):
                            sp = (j - 1) + dw
                            for dhi in range(2):
                                arow[(dwi * 2 + dhi, j)] = \
                                    None if sp == 0 else (row + dhi)
                            if sp == 0:
                                continue
                            t0 = 27 + (dwi * 2) * 3 + j
                            nc.tensor.matmul(
                                rps[:, row:row + 2, :], esh[:, sp + 3, :],
                                amap[:, t0:t0 + 4:3, :],
                                start=True, stop=True, skip_group_check=True)
                            row += 2
                    for dwi2, dw in enumerate((-2, 2)):
                        for j in range(KTAP):
                            sp = (j - 1) + dw
                            t0 = 27 + (6 + dwi2 * 3) * 3 + j
                            nc.tensor.matmul(
                                rps[:, row:row + 3, :], esh[:, sp + 3, :],
                                amap[:, t0:t0 + 7:3, :],
                                start=True, stop=True, skip_group_check=True)
                            for dhi in range(3):
                                arow[(6 + dwi2 * 3 + dhi, j)] = row + dhi
                            row += 3
                    nc.scalar.activation(rashm[:], rps[:], AF.Identity)
                    # ring products + accumulation
                    for ti, (dh, dw) in enumerate(RING):
                        for j in range(KTAP):
                            sp = (j - 1) + dw
                            t = 27 + ti * 3 + j
                            zsrc = zt[j][:, :, HALO + dh:HALO + dh + blk]
                            r = arow[(ti, j)]
                            if r is None:
                                a_b = amap[:, t:t + 1, :] \
                                    .broadcast_to([W, C, blk])
                            else:
                                a_b = rashm[:, r:r + 1, :] \
                                    .broadcast_to([W, C, blk])
                            tmp = tpool.tile([W, C, blk], F16, tag="tmp")
                            nc.vector.tensor_tensor(tmp[:], a_b, zsrc,
                                                    op=OP.mult)
                            lhs = esh[:, 3 - sp, :]
                            for hf in range(2):
                                for cf in range(2):
                                    nc.tensor.matmul(
                                        accs[hf][:, cf * (C // 2):
                                                 (cf + 1) * (C // 2), :],
                                        lhs,
                                        tmp[:, cf * (C // 2):
                                            (cf + 1) * (C // 2),
                                            hf * hb:(hf + 1) * hb],
                                        start=False, stop=True,
                                        skip_group_check=True)
            return accs

        def readout_phase(b, st, accs):
            """Act copies PSUM -> ost [W, blk, C] f32, out DMA."""
            r0 = b * blk
            ost = spool.tile([W, blk, C], F32, tag="ost")
            for hf in range(2):
                nc.scalar.activation(
                    ost[:, hf * hb:(hf + 1) * hb, :],
                    accs[hf][:].transpose([0, 2, 1]), AF.Identity)
            nc.sync.dma_start(out_d[:, r0 * C:(r0 + blk) * C], ost[:])

        sts = {0: maps_phase(0)}
        if nb > 1:
            sts[1] = maps_phase(1)
        zconv_phase(0, sts[0])
        prev = None
        for b in range(nb):
            if b + 2 < nb:
                sts[b + 2] = maps_phase(b + 2)
            if prev is not None:
                readout_phase(*prev)
            if b + 1 < nb:
                zconv_phase(b + 1, sts[b + 1])
            accs = accum_phase(b, sts[b])
            prev = (b, sts.pop(b), accs)
        readout_phase(*prev)
    return nc


def prep_inputs(x, conv_w, conv_b, off_w, off_b, mask_w, mask_b,
                rows=ROWS, ncores=NCORES):
    x = np.asarray(x, np.float32)
    conv_w = np.asarray(conv_w, np.float32)
    # wz[cin, j*C + cout] = conv_w[cout, cin, 0, j]
    wz = np.concatenate([conv_w[:, :, 0, j].T for j in range(KTAP)],
                        axis=1).astype(np.float16)
    wom_t = []
    for t in range(KTAP):
        m = np.concatenate([np.asarray(off_w)[:, :, 0, t],
                            np.asarray(mask_w)[:, :, 0, t]], axis=0)
        wom_t.append(m.T)
    wom = np.concatenate(wom_t, axis=1).astype(np.float16)
    obr = np.concatenate([np.asarray(off_b),
                          np.asarray(mask_b)])[None, :].astype(np.float16)
    b5 = np.tile(np.array([[2.0, 1.0, 0.0, -1.0, -2.0]], np.float32), (W, 1))
    esh = np.stack([np.eye(W, k=k, dtype=np.float16) for k in range(-3, 4)],
                   axis=1).reshape(W, 7 * W)
    ones1 = np.ones((1, W), np.float16)
    cbh = np.repeat(np.asarray(conv_b, np.float32)[None, :, None], 8,
                    axis=2).reshape(1, -1).astype(np.float16)

    xp = np.zeros((B, C, H + 2 * HALO, PITCH), np.float16)
    xp[:, :, HALO:H + HALO, COL0:COL0 + W] = x.astype(np.float16)
    halves = H // rows
    in_maps = []
    for i in range(ncores):
        b, hh = i // halves, i % halves
        xs = np.ascontiguousarray(
            xp[b, :, hh * rows:hh * rows + rows + 2 * HALO, :])
        in_maps.append({"x": xs, "wz": wz, "wom": wom, "obr": obr,
                        "b5": b5, "esh": esh, "ones1": ones1, "cbh": cbh})
    return in_maps


_NC_CACHE = {}


def kernel(x, conv_w, conv_b, off_w, off_b, mask_w, mask_b, **run_kw):
    if "nc" not in _NC_CACHE:
        _NC_CACHE["nc"] = build_nc()
    nc = _NC_CACHE["nc"]
    if not nc.is_finalized():
        nc.finalize()
    in_maps = prep_inputs(x, conv_w, conv_b, off_w, off_b, mask_w, mask_b)
    res = run_bass_kernel_spmd(nc, in_maps, list(range(NCORES)), **run_kw)
    out = np.empty((B, C, H, W), np.float32)
    halves = H // ROWS
    for i in range(NCORES):
        b, hh = i // halves, i % halves
        o = res.results[i]["out"].reshape(W, ROWS, C)
        out[b, :, hh * ROWS:(hh + 1) * ROWS, :] = o.transpose(2, 1, 0)
    _NC_CACHE["last_result"] = res
    return out



# revision 6
# speedup vs baseline: 1.1393x; 1.1393x over previous
"""Deformable temporal conv1d (kernel (1,3), stride 1, pad 1) on 8 TRN2 cores.

v4: batched products + PE-saturating schedule.

Layout: W=128 on partitions, free dims (c, h).  Per-pixel modulation maps
A_t[w, h] broadcast along c via stride-0 APs.  z_j staged as fp16 base
slabs [w, c, srows] aligned at image col 0.  A term (j,dh,dw) needs z at
col w+j-1+dw = base col w+s', s'=j-1+dw in [-2,2]; the partition shift
s' is folded into the PSUM accumulation with lhsT = eye(k=-s').

v4 changes vs v3:
- products batched per (iw, j) group: ONE DVE tensor_tensor covers the
  3 dh terms via an overlapping-window AP on the z slab (4-dim AP
  [w, dh, c, r] with dh-stride 1 == r-stride), writing tmp3 [w,3,c,blk].
- PE stream is kept continuously busy (pstate ramp to 2.4 GHz): zconv
  chunks for block b+1 are interleaved between the accumulation batches
  of block b; om/ashm matmuls for block b+2 are emitted mid-stream
  (after groups 1 and 3) so they never head-of-line block the PE queue.
- conv bias is added on the host after the gather (saves 4 PE matmuls
  per block); output is written fp16.
- ring flag reduce moved to GpSimd/Act/PE (off the critical DVE path).

Sharding: core i handles batch b=i//2, H-half hh=i%2 (256 output rows).
"""

import numpy as np
from contextlib import ExitStack

import concourse.bass as bass
import concourse.bacc as bacc
import concourse.tile as tile
import concourse.mybir as mybir
from concourse.ap import AP
from concourse.bass_utils import run_bass_kernel_spmd

F32 = mybir.dt.float32
F16 = mybir.dt.float16
U32 = mybir.dt.uint32
AF = mybir.ActivationFunctionType
OP = mybir.AluOpType
ET = mybir.EngineType

B, C, H, W = 4, 128, 512, 128
NCORES = 8
ROWS = H // 2          # output rows per core
KTAP = 3
HALO = 2               # halo rows each side
COL0 = 3               # x column offset of image col 0
PITCH = W + 8          # 3 zero cols left, 5 right

# product groups (iw, j), ordered by shift sp = (iw-1)+(j-1) so the PE
# chains same-lhsT runs; each group covers dh in {-1,0,1} in one op.
GROUPS = [(0, 0), (0, 1), (1, 0), (0, 2), (1, 1), (2, 0),
          (1, 2), (2, 1), (2, 2)]
# A-shift groups (iw, j) with s' = (j-1)+(iw-1) != 0
ACOMBO = [(iw, j) for iw in range(3) for j in range(KTAP)
          if (j - 1) + (iw - 1) != 0]
ACI = {c: i for i, c in enumerate(ACOMBO)}
# ring combos, grouped so rows for fixed (dw, j) are amap-stride-3 slices
RING = [(dh, dw) for dw in (-1, 0, 1) for dh in (-2, 2)] + \
       [(dh, dw) for dw in (-2, 2) for dh in (-1, 0, 1)]


def build_nc(rows=ROWS, blk=16, ring=True):
    assert rows % blk == 0
    nb = rows // blk
    srows = blk + 2 * HALO                    # 20
    hb = blk // 2
    Ch = C // 2

    nc = bacc.Bacc()
    x_d = nc.declare_dram_parameter("x", [C, rows + 2 * HALO, PITCH], F16,
                                    isOutput=False)
    wz_d = nc.declare_dram_parameter("wz", [C, KTAP * C], F16, isOutput=False)
    wom_d = nc.declare_dram_parameter("wom", [C, KTAP * 9], F16,
                                      isOutput=False)
    obr_d = nc.declare_dram_parameter("obr", [1, 9], F16, isOutput=False)
    b5_d = nc.declare_dram_parameter("b5", [W, 5], F32, isOutput=False)
    es_d = nc.declare_dram_parameter("esh", [W, 7 * W], F16, isOutput=False)
    ones1_d = nc.declare_dram_parameter("ones1", [1, W], F16, isOutput=False)
    onesc_d = nc.declare_dram_parameter("onesc", [W, 1], F16, isOutput=False)
    out_d = nc.declare_dram_parameter("out", [W, rows * C], F16,
                                      isOutput=True)

    with tile.TileContext(nc) as tc, ExitStack() as ctx:
        cpool = ctx.enter_context(tc.tile_pool(name="consts", bufs=1))
        xpool = ctx.enter_context(tc.tile_pool(name="xs", bufs=3))
        ompool = ctx.enter_context(tc.tile_pool(name="om", bufs=3))
        mpool = ctx.enter_context(tc.tile_pool(name="maps", bufs=3))
        fpool = ctx.enter_context(tc.tile_pool(name="flag", bufs=3))
        tpool = ctx.enter_context(tc.tile_pool(name="tmp", bufs=3))
        spool = ctx.enter_context(tc.tile_pool(name="ostg", bufs=2))
        ps_z = ctx.enter_context(
            tc.tile_pool(name="ps_z", bufs=2, space="PSUM"))
        ps_om = ctx.enter_context(
            tc.tile_pool(name="ps_om", bufs=1, space="PSUM"))
        ps_ash = ctx.enter_context(
            tc.tile_pool(name="ps_ash", bufs=1, space="PSUM"))
        ps_a = ctx.enter_context(
            tc.tile_pool(name="ps_a", bufs=1, space="PSUM"))

        # constants
        wz = cpool.tile([C, KTAP * C], F16, tag="wz")
        nc.sync.dma_start(wz[:], wz_d[:])
        wom = cpool.tile([C, KTAP * 9], F16, tag="wom")
        nc.sync.dma_start(wom[:], wom_d[:])
        obr = cpool.tile([1, 9], F16, tag="obr")
        nc.sync.dma_start(obr[:], obr_d[:])
        # b5 columns hold -dlt for dlt in (-2,-1,0,1,2): (2,1,0,-1,-2)
        b5 = cpool.tile([W, 5], F32, tag="b5")
        nc.sync.dma_start(b5[:], b5_d[:])
        # esh[:, k+3, :] = eye(W, k)
        esh = cpool.tile([W, 7, W], F16, tag="esh")
        nc.sync.dma_start(esh[:].rearrange("p a b -> p (a b)"), es_d[:])
        ones1 = cpool.tile([1, W], F16, tag="ones1")
        nc.sync.dma_start(ones1[:], ones1_d[:])
        onesc = cpool.tile([W, 1], F16, tag="onesc")
        nc.sync.dma_start(onesc[:], onesc_d[:])

        # persistent base z slabs [j] -> [W, C, srows], x2 parity
        zs_bufs = []
        for i in range(2):
            zrow = []
            for j in range(KTAP):
                z = cpool.tile([W, C, srows], F16, tag=f"z{i}_{j}")
                zrow.append(z)
            zs_bufs.append(zrow)

        def maps_dma(b):
            st = {"b": b}
            xs = xpool.tile([C, srows, PITCH], F16, tag="xs")
            nc.sync.dma_start(xs[:], x_d[:, b * blk:b * blk + srows, :])
            st["xs"] = xs
            return st

        def maps_om(st):
            """om conv, activation maps, ring flag (no ashm)."""
            xs = st["xs"]
            # offset/mask conv: om [W, blk, 9] f32 (ob bias via ones row)
            om = ompool.tile([W, blk, 9], F32, tag="om")
            for rq in range(blk // 4):
                ps = ps_om.tile([W, 4, 9], F32, tag="psom")
                for ri in range(4):
                    r = HALO + rq * 4 + ri
                    nc.tensor.matmul(ps[:, ri, :], ones1[:], obr[:],
                                     start=True, stop=False)
                    for t in range(KTAP):
                        nc.tensor.matmul(
                            ps[:, ri, :],
                            xs[:, r, COL0 - 1 + t:COL0 - 1 + t + W],
                            wom[:, t * 9:(t + 1) * 9],
                            start=False, stop=(t == KTAP - 1))
                nc.scalar.activation(om[:, rq * 4:(rq + 1) * 4, :], ps[:],
                                     AF.Identity)

            # maps [W, 3j, blk] f16
            dyv = om[:, :, 0:6:2].transpose([0, 2, 1])
            dxv = om[:, :, 1:7:2].transpose([0, 2, 1])
            mskv = om[:, :, 6:9].transpose([0, 2, 1])
            st["dyv"], st["dxv"] = dyv, dxv
            msk = mpool.tile([W, 3, blk], F16, tag="msk")
            nc.scalar.activation(msk[:], mskv, AF.Sigmoid)
            st["msk"] = msk
            wyall = mpool.tile([W, 3, 3, blk], F16, tag="wyall")
            wxall = mpool.tile([W, 3, 3, blk], F16, tag="wxall")
            wy = {}
            wx = {}
            ay0 = None
            ax0 = None
            for i, (bi, dlt) in enumerate(((1, -1.0), (2, 0.0), (3, 1.0))):
                nbias = b5[:, bi:bi + 1]
                ayt = mpool.tile([W, 3, blk], F16, tag=f"ay{dlt}")
                nc.scalar.activation(ayt[:], dyv, AF.Abs, bias=nbias)
                nc.scalar.activation(wyall[:, i, :, :], ayt[:], AF.Relu,
                                     bias=1.0, scale=-1.0)
                wy[dlt] = wyall[:, i, :, :]
                axt = mpool.tile([W, 3, blk], F16, tag=f"ax{dlt}")
                nc.scalar.activation(axt[:], dxv, AF.Abs, bias=nbias)
                nc.scalar.activation(wxall[:, i, :, :], axt[:], AF.Relu,
                                     bias=1.0, scale=-1.0)
                wx[dlt] = wxall[:, i, :, :]
                if dlt == 0.0:
                    ay0, ax0 = ayt, axt
            st["wy"], st["wx"] = wy, wx

            # ring flag: any |dy|>1 or |dx|>1 in this block?  Off the DVE
            # path: max-combine then full (partition+free) max-reduce on
            # GpSimd; only the tiny is_gt threshold stays on DVE.
            if ring:
                mxf = fpool.tile([W, 3 * blk], F16, tag="mxf")
                nc.vector.tensor_tensor(
                    mxf[:], ay0[:].rearrange("p a b -> p (a b)"),
                    ax0[:].rearrange("p a b -> p (a b)"), op=OP.max)
                rmx = fpool.tile([W, 1], F16, tag="rmx")
                nc.vector.reduce_max(rmx[:], mxf[:],
                                     axis=mybir.AxisListType.X)
                rfl = fpool.tile([W, 1], F32, tag="rfl")
                nc.gpsimd.partition_all_reduce(
                    rfl[:], rmx[:], channels=W,
                    reduce_op=bass.bass_isa.ReduceOp.max)
                rfl01 = fpool.tile([1, 1], F32, tag="rfl01")
                nc.vector.tensor_scalar(rfl01[:], rfl[0:1, :], 1.0, None,
                                        op0=OP.is_gt)
                st["rfl01"] = rfl01

            # A maps [W, 27+36, blk] f16
            amap = mpool.tile([W, 27 + 36, blk], F16, tag="amap")
            st["amap"] = amap
            mywall = mpool.tile([W, 3, 3, blk], F16, tag="mywall")
            nc.vector.tensor_tensor(
                mywall[:], msk[:].unsqueeze(1).broadcast_to([W, 3, 3, blk]),
                wyall[:], op=OP.mult)
            myw = {dh: mywall[:, i, :, :]
                   for i, dh in enumerate((-1.0, 0.0, 1.0))}
            st["myw"] = myw
            for ih in range(3):
                nc.vector.tensor_tensor(
                    amap[:, ih * 9:(ih + 1) * 9, :]
                    .rearrange("p (a b) c -> p a b c", a=3),
                    mywall[:, ih:ih + 1, :, :]
                    .broadcast_to([W, 3, 3, blk]),
                    wxall[:], op=OP.mult)

        def ashm_phase(st):
            """A-shift matmuls: ashm[:, ci, ih, :] = A_(ih,iw,j)[w - s']."""
            amap = st["amap"]
            aps = ps_ash.tile([W, 30, blk], F32, tag="ashp")
            for ci, (iw, j) in enumerate(ACOMBO):
                sp = (j - 1) + (iw - 1)
                t0 = iw * 3 + j
                nc.tensor.matmul(
                    aps[:, 3 * ci:3 * ci + 3, :], esh[:, sp + 3, :],
                    amap[:, t0:t0 + 19:9, :], start=True, stop=True)
            ashm = mpool.tile([W, 6, 3, blk], F16, tag="ashm")
            nc.scalar.activation(
                ashm[:].rearrange("p a b c -> p (a b) c"),
                aps[:, 0:18, :], AF.Identity)
            st["ashm"] = ashm

        def zchunk(st, j, rq):
            """One z-conv chunk: 4 matmuls + fp16 transpose staging."""
            xs = st["xs"]
            zb = st["zt"][j]
            ps = ps_z.tile([W, 4, C], F32, tag="psz")
            for ri in range(4):
                nc.tensor.matmul(
                    ps[:, ri, :], xs[:, rq * 4 + ri, COL0:COL0 + W],
                    wz[:, j * C:(j + 1) * C], start=True, stop=True)
            nc.scalar.activation(
                zb[:, :, rq * 4:rq * 4 + 4],
                ps[:].transpose([0, 2, 1]), AF.Identity)

        def zwin_ap(zb, j):
            """Overlapping window AP [W, dh:3, C, blk] over z slab."""
            sl = zb[:, :, HALO - 1:HALO - 1 + blk]
            return AP(sl.tensor, sl.offset,
                      [[C * srows, W], [1, 3], [srows, C], [1, blk]])

        def compute(b, st, stn, stn2):
            """Accum batches (b) + zconv (b+1) + om/ashm (b+2) interleave."""
            zt = zs_bufs[b % 2]
            st["zt"] = zt
            amap = st["amap"]
            ashm = st.get("ashm")
            chunks = []
            if stn is not None:
                stn["zt"] = zs_bufs[(b + 1) % 2]
                chunks = [(j, rq) for j in range(KTAP)
                          for rq in range(srows // 4)]
            # front chunks keep PE busy while DVE finishes map work
            nfront = min(5, len(chunks))
            for j, rq in chunks[:nfront]:
                zchunk(stn, j, rq)
            ci = nfront

            acc_a = ps_a.tile([W, C, hb], F32, tag="acca")
            acc_b = ps_a.tile([W, C, hb], F32, tag="accb")
            accs = [acc_a, acc_b]
            ng = len(GROUPS)
            for gi, (iw, j) in enumerate(GROUPS):
                sp = (j - 1) + (iw - 1)
                if sp == 0:
                    t0 = iw * 3 + j
                    a_b = amap[:, t0:t0 + 19:9, :].unsqueeze(2) \
                        .broadcast_to([W, 3, C, blk])
                else:
                    a_b = ashm[:, ACI[(iw, j)], :, :].unsqueeze(2) \
                        .broadcast_to([W, 3, C, blk])
                tmp3 = tpool.tile([W, 3, C, blk], F16, tag="tmp3")
                nc.vector.tensor_tensor(tmp3[:], a_b, zwin_ap(zt[j], j),
                                        op=OP.mult)
                lhs = esh[:, 3 - sp, :]
                for dhi in range(3):
                    first = gi == 0 and dhi == 0
                    last = gi == ng - 1 and dhi == 2
                    for hf in range(2):
                        for cf in range(2):
                            nc.tensor.matmul(
                                accs[hf][:, cf * Ch:(cf + 1) * Ch, :], lhs,
                                tmp3[:, dhi, cf * Ch:(cf + 1) * Ch,
                                     hf * hb:(hf + 1) * hb],
                                start=first, stop=last)
                if ci < len(chunks):
                    zchunk(stn, *chunks[ci])
                    ci += 1
                if gi == 1 and stn2 is not None:
                    maps_om(stn2)
                if gi == 3 and stn2 is not None:
                    ashm_phase(stn2)
            while ci < len(chunks):
                zchunk(stn, *chunks[ci])
                ci += 1

            # ring pass (rare): 36 extra terms, If-gated (PE/Act/DVE)
            if ring:
                dyv, dxv = st["dyv"], st["dxv"]
                msk, wy, wx, myw = st["msk"], st["wy"], st["wx"], st["myw"]
                flag_regs = []
                for et in (ET.PE, ET.Activation, ET.DVE):
                    eng = nc.engines[et]
                    r = eng.alloc_register(f"ringflag{b}")
                    eng.reg_load(r, st["rfl01"][:].bitcast(U32))
                    flag_regs.append(r)
                cond = nc.snap(bass.RegisterHandles(flag_regs), donate=True)
                with tc.If(cond != 0):
                    for bi, dlt in ((0, -2.0), (4, 2.0)):
                        nbias = b5[:, bi:bi + 1]
                        ayt = mpool.tile([W, 3, blk], F16, tag=f"ray{dlt}")
                        nc.scalar.activation(ayt[:], dyv, AF.Abs, bias=nbias)
                        wyt = mpool.tile([W, 3, blk], F16, tag=f"rwy{dlt}")
                        nc.scalar.activation(wyt[:], ayt[:], AF.Relu,
                                             bias=1.0, scale=-1.0)
                        wy[dlt] = wyt
                        axt = mpool.tile([W, 3, blk], F16, tag=f"rax{dlt}")
                        nc.scalar.activation(axt[:], dxv, AF.Abs, bias=nbias)
                        wxt = mpool.tile([W, 3, blk], F16, tag=f"rwx{dlt}")
                        nc.scalar.activation(wxt[:], axt[:], AF.Relu,
                                             bias=1.0, scale=-1.0)
                        wx[dlt] = wxt
                        mywt = mpool.tile([W, 3, blk], F16, tag=f"rmyw{dlt}")
                        nc.vector.tensor_tensor(mywt[:], msk[:], wy[dlt][:],
                                                op=OP.mult)
                        myw[dlt] = mywt
                    for ti, (dh, dw) in enumerate(RING):
                        t3 = 27 + ti * 3
                        nc.vector.tensor_tensor(
                            amap[:, t3:t3 + 3, :], myw[float(dh)][:],
                            wx[float(dw)][:], op=OP.mult)
                    # ring A-shifts
                    rps = ps_ash.tile([W, 30, blk], F32, tag="ashp")
                    rashm = mpool.tile([W, 30, blk], F16, tag="rashm")
                    arow = {}
                    row = 0
                    for dwi, dw in enumerate((-1, 0, 1)):
                        for j in range(KTAP):
                            sp = (j - 1) + dw
                            for dhi in range(2):
                                arow[(dwi * 2 + dhi, j)] = \
                                    None if sp == 0 else (row + dhi)
                            if sp == 0:
                                continue
                            t0 = 27 + (dwi * 2) * 3 + j
                            nc.tensor.matmul(
                                rps[:, row:row + 2, :], esh[:, sp + 3, :],
                                amap[:, t0:t0 + 4:3, :],
                                start=True, stop=True, skip_group_check=True)
                            row += 2
                    for dwi2, dw in enumerate((-2, 2)):
                        for j in range(KTAP):
                            sp = (j - 1) + dw
                            t0 = 27 + (6 + dwi2 * 3) * 3 + j
                            nc.tensor.matmul(
                                rps[:, row:row + 3, :], esh[:, sp + 3, :],
                                amap[:, t0:t0 + 7:3, :],
                                start=True, stop=True, skip_group_check=True)
                            for dhi in range(3):
                                arow[(6 + dwi2 * 3 + dhi, j)] = row + dhi
                            row += 3
                    nc.scalar.activation(rashm[:], rps[:], AF.Identity)
                    # ring products + accumulation
                    for ti, (dh, dw) in enumerate(RING):
                        for j in range(KTAP):
                            sp = (j - 1) + dw
                            t = 27 + ti * 3 + j
                            zsrc = zt[j][:, :, HALO + dh:HALO + dh + blk]
                            r = arow[(ti, j)]
                            if r is None:
                                a_b = amap[:, t:t + 1, :] \
                                    .broadcast_to([W, C, blk])
                            else:
                                a_b = rashm[:, r:r + 1, :] \
                                    .broadcast_to([W, C, blk])
                            tmp = tpool.tile([W, C, blk], F16, tag="tmp")
                            nc.vector.tensor_tensor(tmp[:], a_b, zsrc,
                                                    op=OP.mult)
                            lhs = esh[:, 3 - sp, :]
                            for hf in range(2):
                                for cf in range(2):
                                    nc.tensor.matmul(
                                        accs[hf][:, cf * Ch:(cf + 1) * Ch, :],
                                        lhs,
                                        tmp[:, cf * Ch:(cf + 1) * Ch,
                                            hf * hb:(hf + 1) * hb],
                                        start=False, stop=True,
                                        skip_group_check=True)
            return accs

        def readout_phase(b, st, accs):
            """Act copies PSUM -> ost [W, blk, C] f16, out DMA."""
            r0 = b * blk
            ost = spool.tile([W, blk, C], F16, tag="ost")
            for hf in range(2):
                nc.scalar.activation(
                    ost[:, hf * hb:(hf + 1) * hb, :],
                    accs[hf][:].transpose([0, 2, 1]), AF.Identity)
            nc.sync.dma_start(out_d[:, r0 * C:(r0 + blk) * C], ost[:])

        # prologue: maps for blocks 0,1 and full zconv for block 0
        sts = {0: maps_dma(0)}
        if nb > 1:
            sts[1] = maps_dma(1)
        maps_om(sts[0])
        ashm_phase(sts[0])
        if nb > 1:
            maps_om(sts[1])
            ashm_phase(sts[1])
        sts[0]["zt"] = zs_bufs[0]
        for j in range(KTAP):
            for rq in range(srows // 4):
                zchunk(sts[0], j, rq)
        prev = None
        for b in range(nb):
            if b + 2 < nb:
                sts[b + 2] = maps_dma(b + 2)
            if prev is not None:
                readout_phase(*prev)
            accs = compute(b, sts[b], sts.get(b + 1), sts.get(b + 2))
            prev = (b, sts.pop(b), accs)
        readout_phase(*prev)
    return nc


def prep_inputs(x, conv_w, conv_b, off_w, off_b, mask_w, mask_b,
                rows=ROWS, ncores=NCORES):
    x = np.asarray(x, np.float32)
    conv_w = np.asarray(conv_w, np.float32)
    # wz[cin, j*C + cout] = conv_w[cout, cin, 0, j]
    wz = np.concatenate([conv_w[:, :, 0, j].T for j in range(KTAP)],
                        axis=1).astype(np.float16)
    wom_t = []
    for t in range(KTAP):
        m = np.concatenate([np.asarray(off_w)[:, :, 0, t],
                            np.asarray(mask_w)[:, :, 0, t]], axis=0)
        wom_t.append(m.T)
    wom = np.concatenate(wom_t, axis=1).astype(np.float16)
    obr = np.concatenate([np.asarray(off_b),
                          np.asarray(mask_b)])[None, :].astype(np.float16)
    b5 = np.tile(np.array([[2.0, 1.0, 0.0, -1.0, -2.0]], np.float32), (W, 1))
    esh = np.stack([np.eye(W, k=k, dtype=np.float16) for k in range(-3, 4)],
                   axis=1).reshape(W, 7 * W)
    ones1 = np.ones((1, W), np.float16)
    onesc = np.ones((W, 1), np.float16)

    xp = np.zeros((B, C, H + 2 * HALO, PITCH), np.float16)
    xp[:, :, HALO:H + HALO, COL0:COL0 + W] = x.astype(np.float16)
    halves = H // rows
    in_maps = []
    for i in range(ncores):
        b, hh = i // halves, i % halves
        xs = np.ascontiguousarray(
            xp[b, :, hh * rows:hh * rows + rows + 2 * HALO, :])
        in_maps.append({"x": xs, "wz": wz, "wom": wom, "obr": obr,
                        "b5": b5, "esh": esh, "ones1": ones1,
                        "onesc": onesc})
    return in_maps


_NC_CACHE = {}


def kernel(x, conv_w, conv_b, off_w, off_b, mask_w, mask_b, **run_kw):
    if "nc" not in _NC_CACHE:
        _NC_CACHE["nc"] = build_nc()
    nc = _NC_CACHE["nc"]
    if not nc.is_finalized():
        nc.finalize()
    in_maps = prep_inputs(x, conv_w, conv_b, off_w, off_b, mask_w, mask_b)
    res = run_bass_kernel_spmd(nc, in_maps, list(range(NCORES)), **run_kw)
    out = np.empty((B, C, H, W), np.float32)
    halves = H // ROWS
    for i in range(NCORES):
        b, hh = i // halves, i % halves
        o = res.results[i]["out"].reshape(W, ROWS, C).astype(np.float32)
        out[b, :, hh * ROWS:(hh + 1) * ROWS, :] = o.transpose(2, 1, 0)
    out += np.asarray(conv_b, np.float32)[None, :, None, None]
    _NC_CACHE["last_result"] = res
    return out
